# revision 1
# baseline (speedup 1.0000x reference)
"""Trainium2 Bass kernel for nn_Discriminator (MoE-routing discriminator).

Strategy (8 NeuronCores, single SPMD NEFF launch):
  Phase A (expert layer, column-parallel): every core reads ALL samples'
    inputs but only a 128-column slice of every expert's W_in. Core c
    computes hT_c = lrelu(W[:, c*128:(c+1)*128].T @ x.T + b) for all
    (bucketed-by-expert) samples -> [128 features, NT samples].
    This splits the dominant weight traffic (~92 MB of used W_in rows)
    exactly 8 ways with zero duplication. W and x streams alternate
    between the two hardware DGE queues (sync + scalar engines).
  AllToAll x2: feature-sharded hT -> sample-sharded hT. Experts 0-5
    finish early (49 of 180 k-tiles), so their columns exchange in an
    early AllToAll that overlaps the long expert-6 tail; expert 6's
    columns go in a small second AllToAll at the end. The second one is
    shipped sample-major (PE-transposed on both ends) so its DMA staging
    moves 256B lines instead of 40B lines (HW DGE queues are
    packet-rate-bound at ~14ns/line).
  Phase B (shared fc stack, data-parallel): each core runs the 3-layer
    fc stack for its sample shard, keeping activations transposed
    (features on partitions) so no transposes are ever needed. The
    experts-0-5 chunk of phase B hides inside phase A's DMA gaps.

  All matmuls run in bf16 (inputs rounded host-side) with fp32 PSUM
  accumulation; biases/activations applied in fp32.
"""
import os
import ml_dtypes
import numpy as np
from contextlib import ExitStack

import concourse.bacc as bacc
import concourse.bass as bass
import concourse.tile as tile
from concourse import mybir
from concourse.masks import make_identity
from concourse.tile_rust import add_dep_helper
from concourse.bass_utils import run_bass_kernel_spmd

P = 128
NCORES = 8
EMBED_DIM = 16
HIDDEN = 256
N_EXPERTS = 7
SIZES = [(2 ** (o + 1) + 1) ** 2 for o in range(N_EXPERTS)]  # 9..16641
S_MAX = SIZES[-1]
H4 = 4 * HIDDEN   # 1024
H2 = 2 * HIDDEN   # 512
H1 = HIDDEN       # 256
BF16 = mybir.dt.bfloat16
F32 = mybir.dt.float32
GRP = 32  # k-tiles per DMA batch
PRE_GROUPS = 2  # expert-6 groups buffered before the AllToAll-1 staging

_CACHE = {}
last_run = None
DEBUG_OUTPUTS = False


def _round_up(x, m):
    return (x + m - 1) // m * m


def _part_major(a, ktiles, width):
    """[ktiles*128, width] -> [128, ktiles*width] partition-major layout."""
    return np.ascontiguousarray(
        a.reshape(ktiles, P, width).transpose(1, 0, 2).reshape(P, ktiles * width)
    )


def build_program(n_pads):
    """Build the SPMD Bass program. n_pads: per-expert padded sample counts."""
    ktiles = [_round_up(s + EMBED_DIM, P) // P for s in SIZES]
    T = sum(ktiles)
    NT1 = sum(n_pads[:-1])       # experts 0-5 columns (first AllToAll)
    NT2 = n_pads[-1]             # expert 6 columns (second AllToAll)
    G1, G2 = NT1 // NCORES, NT2 // NCORES

    nc = bacc.Bacc("TRN2", target_bir_lowering=False, debug=False,
                   num_devices=NCORES)

    xt_ps = [
        nc.declare_dram_parameter(f"xt{o}", [P, ktiles[o] * n_pads[o]], BF16,
                                  isOutput=False)
        for o in range(N_EXPERTS)
    ]
    wt_p = nc.declare_dram_parameter("wt", [P, T * P], BF16, isOutput=False)
    w1_p = nc.declare_dram_parameter("w1", [P, 8 * H2], BF16, isOutput=False)
    w2_p = nc.declare_dram_parameter("w2", [P, 4 * H1], BF16, isOutput=False)
    w3_p = nc.declare_dram_parameter("w3", [P, 2 * 1], BF16, isOutput=False)
    bin_p = nc.declare_dram_parameter("bin", [P, N_EXPERTS], F32, isOutput=False)
    b1_p = nc.declare_dram_parameter("b1", [P, 4], F32, isOutput=False)
    b2_p = nc.declare_dram_parameter("b2", [P, 2], F32, isOutput=False)
    b3_p = nc.declare_dram_parameter("b3", [1, 1], F32, isOutput=False)
    out_p = nc.declare_dram_parameter("out", [1, G1 + G2], F32, isOutput=True)

    LR = mybir.ActivationFunctionType.Prelu
    SIG = mybir.ActivationFunctionType.Sigmoid
    CP = mybir.ActivationFunctionType.Copy

    with tile.TileContext(nc) as tc, ExitStack() as ctx:
        wpool = ctx.enter_context(tc.tile_pool(name="wpool", bufs=4))
        xpool = ctx.enter_context(tc.tile_pool(name="xpool", bufs=4))
        hpool = ctx.enter_context(tc.tile_pool(name="hpool", bufs=1))
        cpool = ctx.enter_context(tc.tile_pool(name="cpool", bufs=1))
        pspool = ctx.enter_context(tc.tile_pool(name="pspool", bufs=1, space="PSUM"))
        drpool = ctx.enter_context(tc.tile_pool(name="drpool", bufs=1, space="DRAM"))

        # per-engine DMA issue-order chain: force the scheduler to keep the
        # hardware DGE queue order exactly as written (its cost model does
        # not know about per-queue packet-rate limits)
        qlast = {}

        def qdma(eng, out, in_):
            h = eng.dma_start(out, in_)
            key = id(eng)
            if key in qlast:
                add_dep_helper(h.ins, qlast[key].ins, sync=False,
                               reason="hw queue order")
            qlast[key] = h
            return h

        # small constants on the gpsimd (software) queue so the two hardware
        # queues start streaming phase-A data immediately
        binsb = cpool.tile([P, N_EXPERTS], F32)
        nc.gpsimd.dma_start(binsb[:], bin_p[:])
        b1sb = cpool.tile([P, 4], F32)
        nc.gpsimd.dma_start(b1sb[:], b1_p[:])
        b2sb = cpool.tile([P, 2], F32)
        nc.gpsimd.dma_start(b2sb[:], b2_p[:])
        b3sb = cpool.tile([1, 1], F32)
        nc.gpsimd.dma_start(b3sb[:], b3_p[:])
        ident = cpool.tile([P, P], BF16)
        make_identity(nc, ident[:])
        # dummy sigmoid: forces the act-table pass to load a sigmoid-capable
        # function set up front (it also contains parametric_relu), keeping
        # the final sigmoid's table load off the critical path
        dummy = cpool.tile([1, 1], F32)
        nc.scalar.activation(dummy[:], b3sb[:], SIG)
        # fc weights on the gpsimd software queue: off the HW queues entirely,
        # plenty of time before phase B needs them
        w1sb = cpool.tile([P, 8 * H2], BF16)
        nc.gpsimd.dma_start(w1sb[:], w1_p[:])
        w2sb = cpool.tile([P, 4 * H1], BF16)
        nc.gpsimd.dma_start(w2sb[:], w2_p[:])
        w3sb = cpool.tile([P, 2], BF16)
        nc.gpsimd.dma_start(w3sb[:], w3_p[:])

        # ---------------- Phase A: expert layer (column slice) ----------------
        H1t = hpool.tile([P, NT1], BF16)
        H2t = hpool.tile([P, NT2], BF16)
        qtoggle = 0

        def expert_groups(o, base_t, ps, g_from, g_to):
            nonlocal qtoggle
            kt, npad = ktiles[o], n_pads[o]
            for g0 in range(g_from, min(g_to, kt), GRP):
                gcnt = min(GRP, kt - g0)
                weng = nc.sync if qtoggle % 2 == 0 else nc.scalar
                xeng = nc.scalar if qtoggle % 2 == 0 else nc.sync
                qtoggle += 1
                wg = wpool.tile([P, gcnt * P], BF16, tag="wg", name="wg")
                qdma(weng,
                    wg[:, :gcnt * P],
                    wt_p[:, (base_t + g0) * P:(base_t + g0 + gcnt) * P])
                xg = xpool.tile([P, gcnt * npad], BF16, tag="xg", name="xg")
                qdma(xeng,
                    xg[:, :gcnt * npad],
                    xt_ps[o][:, g0 * npad:(g0 + gcnt) * npad])
                for gi in range(gcnt):
                    g = g0 + gi
                    nc.tensor.matmul(
                        ps[:], wg[:, gi * P:(gi + 1) * P],
                        xg[:, gi * npad:(gi + 1) * npad],
                        start=(g == 0), stop=(g == kt - 1))

        def expert_layer(o, base_t, off):
            kt, npad = ktiles[o], n_pads[o]
            ps = pspool.tile([P, npad], F32, tag="psA", padded_shape=[P, 512],
                             bufs=3, name=f"psA{o}")
            expert_groups(o, base_t, ps, 0, kt)
            nc.scalar.activation(H1t[:, off:off + npad], ps[:], LR,
                                 bias=binsb[:, o:o + 1], alpha=0.2)

        base_ts = np.cumsum([0] + ktiles[:-1]).tolist()
        offs = np.cumsum([0] + n_pads[:-1]).tolist()
        # small experts first: their epilogues wait only briefly, so the
        # scalar engine's DMA trigger stream is never blocked for long
        for o in (0, 1, 2, 3, 4):
            expert_layer(o, base_ts[o], offs[o])
        # expert 5 (34 k-tiles): issue its groups, but defer its epilogue
        # until after expert 6's first groups are buffered in the queues
        ps5 = pspool.tile([P, n_pads[5]], F32, tag="psA", padded_shape=[P, 512],
                          bufs=3, name="psA5")
        expert_groups(5, base_ts[5], ps5, 0, ktiles[5])
        base_t = base_ts[N_EXPERTS - 1]

        # -------- expert 6: first buffer a couple of groups into the queues --
        o6 = N_EXPERTS - 1
        ps6 = pspool.tile([P, n_pads[o6]], F32, tag="psA", padded_shape=[P, 512],
                          bufs=3, name="psA6")
        PRE = PRE_GROUPS * GRP
        expert_groups(o6, base_t, ps6, 0, PRE)
        nc.scalar.activation(H1t[:, offs[5]:offs[5] + n_pads[5]], ps5[:], LR,
                             bias=binsb[:, 5:6], alpha=0.2)

        # -------- early AllToAll for experts 0-5 (feature-major wire) -------
        a2a_in1 = drpool.tile([NCORES * P, G1], BF16, name="a2a_in1")
        a2a_out1 = drpool.tile([NCORES * P, G1], BF16, name="a2a_out1")
        half = NCORES // 2
        qdma(nc.sync,
            a2a_in1[:half * P, :].rearrange("(s p) j -> p s j", p=P),
            H1t[:, :half * G1].rearrange("p (s j) -> p s j", s=half))
        qdma(nc.scalar,
            a2a_in1[half * P:, :].rearrange("(s p) j -> p s j", p=P),
            H1t[:, half * G1:].rearrange("p (s j) -> p s j", s=half))

        nc.gpsimd.collective_compute(
            "AllToAll", mybir.AluOpType.bypass,
            ins=[a2a_in1[:]], outs=[a2a_out1[:]],
            replica_groups=[list(range(NCORES))])

        # -------- rest of expert 6 (73% of the k-tiles) ---------------------
        expert_groups(o6, base_t, ps6, PRE, ktiles[o6])
        nc.scalar.activation(H2t[:, :n_pads[o6]], ps6[:], LR,
                             bias=binsb[:, o6:o6 + 1], alpha=0.2)

        # transpose H2t to sample-major for a fat-line wire format
        hs6 = []
        h2chunks = [(0, min(P, NT2))]
        if NT2 > P:
            h2chunks.append((P, NT2 - P))
        for i, (c0, cw) in enumerate(h2chunks):
            pst = pspool.tile([cw, P], BF16, tag="pstr", padded_shape=[P, P],
                              bufs=2, name=f"pstr{i}")
            nc.tensor.transpose(pst[:], H2t[:, c0:c0 + cw], ident[:])
            t = hpool.tile([cw, P], BF16, tag=f"hs6_{i}", name=f"hs6_{i}")
            nc.scalar.activation(t[:], pst[:], CP)
            hs6.append(t)

        a2a_in2 = drpool.tile([NT2, P], BF16, name="a2a_in2")
        a2a_out2 = drpool.tile([NT2, P], BF16, name="a2a_out2")
        for i, (c0, cw) in enumerate(h2chunks):
            qdma(nc.sync if i == 0 else nc.scalar,
                 a2a_in2[c0:c0 + cw, :], hs6[i][:])
        nc.gpsimd.collective_compute(
            "AllToAll", mybir.AluOpType.bypass,
            ins=[a2a_in2[:]], outs=[a2a_out2[:]],
            replica_groups=[list(range(NCORES))])

        # hT1: per-rank feature blocks, straight [128, G1] loads; queued
        # behind the expert-6 stream, executing while AllToAll #2 is in flight
        hT1 = []
        for r in range(NCORES):
            t = hpool.tile([P, G1], BF16, tag="hT1", bufs=NCORES, name=f"hT1_{r}")
            eng = nc.sync if r % 2 == 0 else nc.scalar
            qdma(eng, t[:], a2a_out1[r * P:(r + 1) * P, :])
            hT1.append(t)

        # receive: batched loads (as many ranks as fit in 128 partitions),
        # one PE transpose per batch; the transposed psum's free axis is
        # (rank, sample) row-major which IS hT2's column order
        hT2 = hpool.tile([P, NCORES * G2], BF16)
        rb = max(1, P // G2)          # ranks per batch
        r0 = 0
        bi = 0
        while r0 < NCORES:
            rcnt = min(rb, NCORES - r0)
            rows = rcnt * G2
            st2 = hpool.tile([rows, P], BF16, tag="st2", bufs=2, name=f"st2_{bi}")
            eng = nc.sync if bi % 2 == 0 else nc.scalar
            qdma(eng, st2[:], a2a_out2[r0 * G2:(r0 + rcnt) * G2, :])
            pst = pspool.tile([P, rows], BF16, tag="pstr", padded_shape=[P, P],
                              bufs=2, name=f"pst2_{bi}")
            nc.tensor.transpose(pst[:], st2[:], ident[:rows, :rows])
            nc.scalar.activation(hT2[:, r0 * G2:(r0 + rcnt) * G2], pst[:], CP)
            r0 += rcnt
            bi += 1

        # ---------------- Phase B: fc stack on my sample shard ---------------
        # chunk 1 (experts 0-5 samples) overlaps phase A's expert-6 tail;
        # chunk 2 (expert 6 samples) runs after the second AllToAll.
        def fc_chunk(gs, rhs_of, ocol):
            z1 = hpool.tile([P, 4 * gs], BF16, tag=f"z1_{ocol}", name=f"z1_{ocol}")
            for m in range(4):
                ps1 = pspool.tile([P, gs], F32, tag="psB",
                                  padded_shape=[P, 512], bufs=2, name=f"ps1_{ocol}_{m}")
                for r in range(NCORES):
                    nc.tensor.matmul(
                        ps1[:], w1sb[:, r * H2 + m * P:r * H2 + (m + 1) * P],
                        rhs_of(r),
                        start=(r == 0), stop=(r == NCORES - 1))
                nc.scalar.activation(z1[:, m * gs:(m + 1) * gs], ps1[:], LR,
                                     bias=b1sb[:, m:m + 1], alpha=0.2)

            z2 = hpool.tile([P, 2 * gs], BF16, tag=f"z2_{ocol}", name=f"z2_{ocol}")
            for m in range(2):
                ps2 = pspool.tile([P, gs], F32, tag="psB",
                                  padded_shape=[P, 512], bufs=2, name=f"ps2_{ocol}_{m}")
                for r in range(4):
                    nc.tensor.matmul(
                        ps2[:], w2sb[:, r * H1 + m * P:r * H1 + (m + 1) * P],
                        z1[:, r * gs:(r + 1) * gs],
                        start=(r == 0), stop=(r == 3))
                nc.scalar.activation(z2[:, m * gs:(m + 1) * gs], ps2[:], LR,
                                     bias=b2sb[:, m:m + 1], alpha=0.2)

            ps3 = pspool.tile([1, gs], F32, tag="psC", bufs=1, name=f"ps3_{ocol}")
            for r in range(2):
                nc.tensor.matmul(ps3[:], w3sb[:, r:r + 1],
                                 z2[:, r * gs:(r + 1) * gs],
                                 start=(r == 0), stop=(r == 1))
            osb = hpool.tile([1, gs], F32, tag=f"osb{ocol}", name=f"osb{ocol}")
            nc.scalar.activation(osb[:], ps3[:], SIG, bias=b3sb[:, 0:1])
            qdma(nc.sync, out_p[:, ocol:ocol + gs], osb[:])

        fc_chunk(G1, lambda r: hT1[r][:], 0)
        fc_chunk(G2, lambda r: hT2[:, r * G2:(r + 1) * G2], G1)

    nc.compile()
    return nc


def kernel(mazes, orders, embed_table, W_in, b_in, W1, b1, W2, b2, W3, b3):
    mazes = np.asarray(mazes)
    orders = np.asarray(orders)
    B = mazes.shape[0]

    # ---- sample routing (host) ----
    idx = [np.where(orders == o)[0] for o in range(N_EXPERTS)]
    ns = [len(i) for i in idx]
    n_pads = [max(16, _round_up(n, 16)) for n in ns]
    G1 = sum(n_pads[:-1]) // NCORES
    G2 = n_pads[-1] // NCORES
    ktiles = [_round_up(s + EMBED_DIM, P) // P for s in SIZES]
    T = sum(ktiles)

    # ---- per-expert xT buffers (shared across cores) ----
    emb16 = np.asarray(embed_table, ml_dtypes.bfloat16)
    xts = {}
    for o in range(N_EXPERTS):
        s, kt, npad = SIZES[o], ktiles[o], n_pads[o]
        X = np.zeros((kt * P, npad), ml_dtypes.bfloat16)
        X[:s, :ns[o]] = np.asarray(mazes[idx[o], :s], ml_dtypes.bfloat16).T
        X[s:s + EMBED_DIM, :ns[o]] = emb16[o][:, None]
        xts[f"xt{o}"] = _part_major(X, kt, npad)

    # ---- per-core W_in column slices ----
    W_in = np.asarray(W_in)
    w16 = []
    for o in range(N_EXPERTS):
        s, kt = SIZES[o], ktiles[o]
        Wo = np.zeros((kt * P, H4), ml_dtypes.bfloat16)
        Wo[:s] = W_in[o, :s].astype(ml_dtypes.bfloat16)
        Wo[s:s + EMBED_DIM] = W_in[o, S_MAX:].astype(ml_dtypes.bfloat16)
        w16.append(Wo)
    wts = []
    for c in range(NCORES):
        Wc = np.concatenate([w[:, c * P:(c + 1) * P] for w in w16], axis=0)
        wts.append(_part_major(Wc, T, P))

    # ---- shared fc stack ----
    W1_16 = _part_major(np.asarray(W1, ml_dtypes.bfloat16), 8, H2)
    W2_16 = _part_major(np.asarray(W2, ml_dtypes.bfloat16), 4, H1)
    W3_16 = _part_major(np.asarray(W3, ml_dtypes.bfloat16), 2, 1)
    b1t = np.ascontiguousarray(np.asarray(b1, np.float32).reshape(4, P).T)
    b2t = np.ascontiguousarray(np.asarray(b2, np.float32).reshape(2, P).T)
    b3t = np.asarray(b3, np.float32).reshape(1, 1)
    b_in = np.asarray(b_in, np.float32)

    key = (tuple(n_pads), GRP, PRE_GROUPS)
    if key not in _CACHE:
        _CACHE[key] = build_program(n_pads)
    nc = _CACHE[key]

    in_maps = []
    for c in range(NCORES):
        m = dict(xts)
        m["wt"] = wts[c]
        m["w1"], m["w2"], m["w3"] = W1_16, W2_16, W3_16
        m["bin"] = np.ascontiguousarray(
            np.stack([b_in[o, c * P:(c + 1) * P] for o in range(N_EXPERTS)], 1))
        m["b1"], m["b2"], m["b3"] = b1t, b2t, b3t
        in_maps.append(m)

    trace = os.environ.get("KERNEL_TRACE") == "1"
    res = run_bass_kernel_spmd(nc, in_maps, list(range(NCORES)), trace=trace)
    global last_run
    last_run = res

    allc = np.stack([res.results[c]["out"][0] for c in range(NCORES)])  # [8, G1+G2]
    half1 = allc[:, :G1].reshape(-1)   # experts 0-5 padded samples
    half2 = allc[:, G1:].reshape(-1)   # expert 6 padded samples

    full = np.zeros((B, 1), np.float32)
    offs = np.cumsum([0] + n_pads[:-2])
    for o in range(N_EXPERTS - 1):
        full[idx[o], 0] = half1[offs[o]:offs[o] + ns[o]]
    full[idx[N_EXPERTS - 1], 0] = half2[:ns[N_EXPERTS - 1]]
    return full



# revision 5
# speedup vs baseline: 1.1799x; 1.1799x over previous
"""Trainium2 Bass kernel for nn_Discriminator (MoE-routing discriminator).

Strategy (8 NeuronCores, single SPMD NEFF launch):
  Phase A (expert layer, column-parallel): every core reads ALL samples'
    inputs but only a 128-column slice of every expert's W_in. Core c
    computes hT_c = lrelu(W[:, c*128:(c+1)*128].T @ x.T + b) for all
    (bucketed-by-expert) samples -> [128 features, NT samples].
    W and x are shipped as fp8e4 (W pre-scaled x16, descale folded into
    the epilogue activation scale) and the matmuls run DoubleRow
    (256-deep contraction per pass): this halves both the HBM traffic
    (the dominant cost) and the PE time vs bf16.
  AllToAll x2 turns the feature-sharded hT into sample-sharded hT.
    Experts 0-5 finish within the first ~30% of phase A, so their
    AllToAll is staged (via the otherwise-idle GpSimd SWDGE, NOT the
    flow-controlled sync/scalar HW queues) and triggered (from the
    otherwise-idle Vector engine) early, completing inside phase A.
    Expert 6's AllToAll goes right after its epilogue. A tiny dummy
    AllToAll fires at t=0 to absorb the collective firmware's one-time
    wake-up latency off the critical path.
  Phase B (shared fc stack, data-parallel): each core runs the 3-layer
    fc stack for its sample shard, keeping activations transposed
    (features on partitions) so no transposes are ever needed. The
    experts-0-5 chunk of phase B hides inside phase A's expert-6 tail.

  Engine roles: sync = phase-A W stream + receives + output DMA;
  scalar = phase-A x stream + all activations; gpsimd = small consts +
  collective staging + fc weights; vector = collective triggers;
  tensor = matmuls + transposes.
"""
import os
import ml_dtypes
import numpy as np
from contextlib import ExitStack

import concourse.bacc as bacc
import concourse.bass as bass
import concourse.tile as tile
from concourse import mybir
from concourse.masks import make_identity
from concourse.tile_rust import add_dep_helper
from concourse.bass_utils import run_bass_kernel_spmd

P = 128
NCORES = 8
EMBED_DIM = 16
HIDDEN = 256
N_EXPERTS = 7
SIZES = [(2 ** (o + 1) + 1) ** 2 for o in range(N_EXPERTS)]  # 9..16641
S_MAX = SIZES[-1]
H4 = 4 * HIDDEN   # 1024
H2 = 2 * HIDDEN   # 512
H1 = HIDDEN       # 256
BF16 = mybir.dt.bfloat16
F32 = mybir.dt.float32
FP8 = mybir.dt.float8e4
NPFP8 = ml_dtypes.float8_e4m3fn
WSCALE = 16.0     # W_in pre-scale before fp8 cast; descaled in epilogue
GRP = 24          # k-tile PAIRS per DMA batch (48 k-tiles)

_CACHE = {}
last_run = None


def _round_up(x, m):
    return (x + m - 1) // m * m


def _part_major(a, ktiles, width):
    """[ktiles*128, width] -> [128, ktiles*width] partition-major layout."""
    return np.ascontiguousarray(
        a.reshape(ktiles, P, width).transpose(1, 0, 2).reshape(P, ktiles * width)
    )


def _ktiles2(s):
    """k-tiles for payload s+EMBED_DIM, rounded up to an even count."""
    return _round_up(_round_up(s + EMBED_DIM, P) // P, 2)


def build_program(n_pads):
    """Build the SPMD Bass program. n_pads: per-expert padded sample counts."""
    ktiles = [_ktiles2(s) for s in SIZES]   # even per expert
    pairs = [k // 2 for k in ktiles]
    T = sum(ktiles)
    NT1 = sum(n_pads[:-1])       # experts 0-5 columns (first AllToAll)
    NT2 = n_pads[-1]             # expert 6 columns (second AllToAll)
    G1, G2 = NT1 // NCORES, NT2 // NCORES

    nc = bacc.Bacc("TRN2", target_bir_lowering=False, debug=False,
                   num_devices=NCORES)

    xt_ps = [
        nc.declare_dram_parameter(f"xt{o}", [P, ktiles[o] * n_pads[o]], FP8,
                                  isOutput=False)
        for o in range(N_EXPERTS)
    ]
    wt_p = nc.declare_dram_parameter("wt", [P, T * P], FP8, isOutput=False)
    w1_p = nc.declare_dram_parameter("w1", [P, 8 * H2], BF16, isOutput=False)
    w2_p = nc.declare_dram_parameter("w2", [P, 4 * H1], BF16, isOutput=False)
    w3_p = nc.declare_dram_parameter("w3", [P, 2 * 1], BF16, isOutput=False)
    bin_p = nc.declare_dram_parameter("bin", [P, N_EXPERTS], F32, isOutput=False)
    b1_p = nc.declare_dram_parameter("b1", [P, 4], F32, isOutput=False)
    b2_p = nc.declare_dram_parameter("b2", [P, 2], F32, isOutput=False)
    b3_p = nc.declare_dram_parameter("b3", [1, 1], F32, isOutput=False)
    out_p = nc.declare_dram_parameter("out", [1, G1 + G2], F32, isOutput=True)

    LR = mybir.ActivationFunctionType.Prelu
    SIG = mybir.ActivationFunctionType.Sigmoid
    CP = mybir.ActivationFunctionType.Copy
    DR = mybir.MatmulPerfMode.DoubleRow
    INV = 1.0 / WSCALE

    with tile.TileContext(nc) as tc, ExitStack() as ctx:
        wpool = ctx.enter_context(tc.tile_pool(name="wpool", bufs=3))
        xpool = ctx.enter_context(tc.tile_pool(name="xpool", bufs=3))
        hpool = ctx.enter_context(tc.tile_pool(name="hpool", bufs=1))
        cpool = ctx.enter_context(tc.tile_pool(name="cpool", bufs=1))
        pspool = ctx.enter_context(tc.tile_pool(name="pspool", bufs=1, space="PSUM"))
        drpool = ctx.enter_context(tc.tile_pool(name="drpool", bufs=1, space="DRAM"))

        # per-engine DMA issue-order chain for the two HW DGE queues
        qlast = {}

        def qdma(eng, out, in_):
            h = eng.dma_start(out, in_)
            key = id(eng)
            if key in qlast:
                add_dep_helper(h.ins, qlast[key].ins, sync=False,
                               reason="hw queue order")
            qlast[key] = h
            return h

        # ---- small constants + collective warm-up, all off the HW queues ----
        binsb = cpool.tile([P, N_EXPERTS], F32)
        nc.gpsimd.dma_start(binsb[:], bin_p[:])
        b1sb = cpool.tile([P, 4], F32)
        nc.gpsimd.dma_start(b1sb[:], b1_p[:])
        b2sb = cpool.tile([P, 2], F32)
        nc.gpsimd.dma_start(b2sb[:], b2_p[:])
        b3sb = cpool.tile([1, 1], F32)
        nc.gpsimd.dma_start(b3sb[:], b3_p[:])

        # dummy AllToAll: wakes the collective firmware while phase A streams
        dum_sb = cpool.tile([NCORES, 16], F32)
        nc.gpsimd.memset(dum_sb[:], 0.0)
        dum_in = drpool.tile([NCORES, 16], F32, name="dum_in")
        dum_out = drpool.tile([NCORES, 16], F32, name="dum_out")
        nc.gpsimd.dma_start(dum_in[:], dum_sb[:])
        nc.gpsimd.collective_compute(
            "AllToAll", mybir.AluOpType.bypass,
            ins=[dum_in[:]], outs=[dum_out[:]],
            replica_groups=[list(range(NCORES))])

        ident = cpool.tile([P, P], BF16)
        make_identity(nc, ident[:])
        # dummy sigmoid: preload the act table set containing sigmoid+prelu
        dummy = cpool.tile([1, 1], F32)
        nc.scalar.activation(dummy[:], b3sb[:], SIG)
        # fc weights (declared here, DMA'd on gpsimd after the first
        # AllToAll is staged+triggered: done by ~18us, needed at ~25us)
        w1sb = cpool.tile([P, 8 * H2], BF16)
        w2sb = cpool.tile([P, 4 * H1], BF16)
        w3sb = cpool.tile([P, 2], BF16)

        # ---------------- Phase A: expert layer (column slice) ----------------
        H1t = hpool.tile([P, NT1], BF16)
        H2t = hpool.tile([P, NT2], BF16)
        qtoggle = 0

        def expert_layer(o, base_t, off, Ht):
            nonlocal qtoggle
            pr, npad = pairs[o], n_pads[o]
            ps = pspool.tile([P, npad], F32, tag="psA", padded_shape=[P, 512],
                             bufs=3, name=f"psA{o}")
            for g0 in range(0, pr, GRP):
                gcnt = min(GRP, pr - g0)
                weng = nc.sync if qtoggle % 2 == 0 else nc.scalar
                xeng = nc.scalar if qtoggle % 2 == 0 else nc.sync
                qtoggle += 1
                wg = wpool.tile([P, gcnt * 2 * P], FP8, tag="wg", name="wg")
                qdma(weng,
                    wg[:, :gcnt * 2 * P],
                    wt_p[:, (base_t + 2 * g0) * P:(base_t + 2 * (g0 + gcnt)) * P])
                xg = xpool.tile([P, gcnt * 2 * npad], FP8, tag="xg", name="xg")
                qdma(xeng,
                    xg[:, :gcnt * 2 * npad],
                    xt_ps[o][:, 2 * g0 * npad:2 * (g0 + gcnt) * npad])
                for gi in range(gcnt):
                    g = g0 + gi
                    nc.tensor.matmul(
                        ps[:],
                        wg[:, gi * 2 * P:(gi + 1) * 2 * P].rearrange(
                            "p (two m) -> p two m", two=2),
                        xg[:, gi * 2 * npad:(gi + 1) * 2 * npad].rearrange(
                            "p (two n) -> p two n", two=2),
                        start=(g == 0), stop=(g == pr - 1), perf_mode=DR)
            nc.scalar.activation(Ht[:, off:off + npad], ps[:], LR,
                                 bias=binsb[:, o:o + 1], scale=INV, alpha=0.2)

        base_ts = np.cumsum([0] + ktiles[:-1]).tolist()
        offs = np.cumsum([0] + n_pads[:-1]).tolist()
        for o in range(N_EXPERTS - 1):
            expert_layer(o, base_ts[o], offs[o], H1t)

        # -------- early AllToAll for experts 0-5, staged via gpsimd SWDGE ----
        a2a_in1 = drpool.tile([NCORES * P, G1], BF16, name="a2a_in1")
        a2a_out1 = drpool.tile([NCORES * P, G1], BF16, name="a2a_out1")
        nc.gpsimd.dma_start(
            a2a_in1[:].rearrange("(s p) j -> p s j", p=P),
            H1t[:].rearrange("p (s j) -> p s j", s=NCORES))
        nc.gpsimd.collective_compute(
            "AllToAll", mybir.AluOpType.bypass,
            ins=[a2a_in1[:]], outs=[a2a_out1[:]],
            replica_groups=[list(range(NCORES))])
        # fc weights now, on the same gpsimd SWDGE queue (behind the
        # time-critical staging+trigger, ahead of anything blocking)
        nc.gpsimd.dma_start(w1sb[:], w1_p[:])
        nc.gpsimd.dma_start(w2sb[:], w2_p[:])
        nc.gpsimd.dma_start(w3sb[:], w3_p[:])

        # -------- expert 6 (73% of the k-tiles) -----------------------------
        o6 = N_EXPERTS - 1
        expert_layer(o6, base_ts[o6], 0, H2t)

        # hT1 receive: one batched 3D-AP load on the (now idle) sync queue
        hT1sb = hpool.tile([P, NCORES * G1], BF16)
        qdma(nc.sync,
             hT1sb[:].rearrange("p (s j) -> p s j", s=NCORES),
             a2a_out1[:].rearrange("(s p) j -> p s j", p=P))

        # transpose H2t to sample-major for the second AllToAll's wire format
        hs6 = []
        h2chunks = [(0, min(P, NT2))]
        if NT2 > P:
            h2chunks.append((P, NT2 - P))
        for i, (c0, cw) in enumerate(h2chunks):
            pst = pspool.tile([cw, P], BF16, tag="pstr", padded_shape=[P, P],
                              bufs=2, name=f"pstr{i}")
            nc.tensor.transpose(pst[:], H2t[:, c0:c0 + cw], ident[:])
            t = hpool.tile([cw, P], BF16, tag=f"hs6_{i}", name=f"hs6_{i}")
            nc.scalar.activation(t[:], pst[:], CP)
            hs6.append(t)

        a2a_in2 = drpool.tile([NT2, P], BF16, name="a2a_in2")
        a2a_out2 = drpool.tile([NT2, P], BF16, name="a2a_out2")
        for i, (c0, cw) in enumerate(h2chunks):
            nc.gpsimd.dma_start(a2a_in2[c0:c0 + cw, :], hs6[i][:])
        nc.gpsimd.collective_compute(
            "AllToAll", mybir.AluOpType.bypass,
            ins=[a2a_in2[:]], outs=[a2a_out2[:]],
            replica_groups=[list(range(NCORES))])

        # receive: batched loads (as many ranks as fit in 128 partitions),
        # one PE transpose per batch; the transposed psum's free axis is
        # (rank, sample) row-major which IS hT2's column order
        hT2 = hpool.tile([P, NCORES * G2], BF16)
        rb = max(1, P // G2)          # ranks per batch
        r0 = 0
        bi = 0
        while r0 < NCORES:
            rcnt = min(rb, NCORES - r0)
            rows = rcnt * G2
            st2 = hpool.tile([rows, P], BF16, tag="st2", bufs=2, name=f"st2_{bi}")
            qdma(nc.sync, st2[:], a2a_out2[r0 * G2:(r0 + rcnt) * G2, :])
            pst = pspool.tile([P, rows], BF16, tag="pstr", padded_shape=[P, P],
                              bufs=2, name=f"pst2_{bi}")
            nc.tensor.transpose(pst[:], st2[:], ident[:rows, :rows])
            nc.scalar.activation(hT2[:, r0 * G2:(r0 + rcnt) * G2], pst[:], CP)
            r0 += rcnt
            bi += 1

        # ---------------- Phase B: fc stack on my sample shard ---------------
        # chunk 1 (experts 0-5 samples) overlaps phase A's expert-6 tail;
        # chunk 2 (expert 6 samples) runs after the second AllToAll.
        def fc_chunk(gs, rhs_of, ocol):
            z1 = hpool.tile([P, 4 * gs], BF16, tag=f"z1_{ocol}", name=f"z1_{ocol}")
            for m in range(4):
                ps1 = pspool.tile([P, gs], F32, tag="psB",
                                  padded_shape=[P, 512], bufs=2, name=f"ps1_{ocol}_{m}")
                for r in range(NCORES):
                    nc.tensor.matmul(
                        ps1[:], w1sb[:, r * H2 + m * P:r * H2 + (m + 1) * P],
                        rhs_of(r),
                        start=(r == 0), stop=(r == NCORES - 1))
                nc.scalar.activation(z1[:, m * gs:(m + 1) * gs], ps1[:], LR,
                                     bias=b1sb[:, m:m + 1], alpha=0.2)

            z2 = hpool.tile([P, 2 * gs], BF16, tag=f"z2_{ocol}", name=f"z2_{ocol}")
            for m in range(2):
                ps2 = pspool.tile([P, gs], F32, tag="psB",
                                  padded_shape=[P, 512], bufs=2, name=f"ps2_{ocol}_{m}")
                for r in range(4):
                    nc.tensor.matmul(
                        ps2[:], w2sb[:, r * H1 + m * P:r * H1 + (m + 1) * P],
                        z1[:, r * gs:(r + 1) * gs],
                        start=(r == 0), stop=(r == 3))
                nc.scalar.activation(z2[:, m * gs:(m + 1) * gs], ps2[:], LR,
                                     bias=b2sb[:, m:m + 1], alpha=0.2)

            ps3 = pspool.tile([1, gs], F32, tag="psC", bufs=1, name=f"ps3_{ocol}")
            for r in range(2):
                nc.tensor.matmul(ps3[:], w3sb[:, r:r + 1],
                                 z2[:, r * gs:(r + 1) * gs],
                                 start=(r == 0), stop=(r == 1))
            osb = hpool.tile([1, gs], F32, tag=f"osb{ocol}", name=f"osb{ocol}")
            nc.scalar.activation(osb[:], ps3[:], SIG, bias=b3sb[:, 0:1])
            qdma(nc.sync, out_p[:, ocol:ocol + gs], osb[:])

        fc_chunk(G1, lambda r: hT1sb[:, r * G1:(r + 1) * G1], 0)
        fc_chunk(G2, lambda r: hT2[:, r * G2:(r + 1) * G2], G1)

    nc.compile()
    return nc


def kernel(mazes, orders, embed_table, W_in, b_in, W1, b1, W2, b2, W3, b3):
    mazes = np.asarray(mazes)
    orders = np.asarray(orders)
    B = mazes.shape[0]

    # ---- sample routing (host) ----
    idx = [np.where(orders == o)[0] for o in range(N_EXPERTS)]
    ns = [len(i) for i in idx]
    n_pads = [max(16, _round_up(n, 16)) for n in ns]
    G1 = sum(n_pads[:-1]) // NCORES
    G2 = n_pads[-1] // NCORES
    ktiles = [_ktiles2(s) for s in SIZES]
    T = sum(ktiles)

    # ---- per-expert xT buffers (fp8, shared across cores) ----
    emb8 = np.asarray(embed_table, NPFP8)
    xts = {}
    for o in range(N_EXPERTS):
        s, kt, npad = SIZES[o], ktiles[o], n_pads[o]
        X = np.zeros((kt * P, npad), NPFP8)
        X[:s, :ns[o]] = np.asarray(mazes[idx[o], :s], NPFP8).T
        X[s:s + EMBED_DIM, :ns[o]] = emb8[o][:, None]
        xts[f"xt{o}"] = _part_major(X, kt, npad)

    # ---- per-core W_in column slices (scaled x16, fp8) ----
    W_in = np.asarray(W_in)
    w8 = []
    for o in range(N_EXPERTS):
        s, kt = SIZES[o], ktiles[o]
        Wo = np.zeros((kt * P, H4), NPFP8)
        Wo[:s] = np.clip(W_in[o, :s] * WSCALE, -240, 240).astype(NPFP8)
        Wo[s:s + EMBED_DIM] = np.clip(
            W_in[o, S_MAX:] * WSCALE, -240, 240).astype(NPFP8)
        w8.append(Wo)
    wts = []
    for c in range(NCORES):
        Wc = np.concatenate([w[:, c * P:(c + 1) * P] for w in w8], axis=0)
        wts.append(_part_major(Wc, T, P))

    # ---- shared fc stack ----
    W1_16 = _part_major(np.asarray(W1, ml_dtypes.bfloat16), 8, H2)
    W2_16 = _part_major(np.asarray(W2, ml_dtypes.bfloat16), 4, H1)
    W3_16 = _part_major(np.asarray(W3, ml_dtypes.bfloat16), 2, 1)
    b1t = np.ascontiguousarray(np.asarray(b1, np.float32).reshape(4, P).T)
    b2t = np.ascontiguousarray(np.asarray(b2, np.float32).reshape(2, P).T)
    b3t = np.asarray(b3, np.float32).reshape(1, 1)
    b_in = np.asarray(b_in, np.float32)

    key = (tuple(n_pads), GRP)
    if key not in _CACHE:
        _CACHE[key] = build_program(n_pads)
    nc = _CACHE[key]

    in_maps = []
    for c in range(NCORES):
        m = dict(xts)
        m["wt"] = wts[c]
        m["w1"], m["w2"], m["w3"] = W1_16, W2_16, W3_16
        m["bin"] = np.ascontiguousarray(
            np.stack([b_in[o, c * P:(c + 1) * P] for o in range(N_EXPERTS)], 1))
        m["b1"], m["b2"], m["b3"] = b1t, b2t, b3t
        in_maps.append(m)

    trace = os.environ.get("KERNEL_TRACE") == "1"
    res = run_bass_kernel_spmd(nc, in_maps, list(range(NCORES)), trace=trace)
    global last_run
    last_run = res

    allc = np.stack([res.results[c]["out"][0] for c in range(NCORES)])  # [8, G1+G2]
    half1 = allc[:, :G1].reshape(-1)   # experts 0-5 padded samples
    half2 = allc[:, G1:].reshape(-1)   # expert 6 padded samples

    full = np.zeros((B, 1), np.float32)
    offs = np.cumsum([0] + n_pads[:-2])
    for o in range(N_EXPERTS - 1):
        full[idx[o], 0] = half1[offs[o]:offs[o] + ns[o]]
    full[idx[N_EXPERTS - 1], 0] = half2[:ns[N_EXPERTS - 1]]
    return full
